# revision 23
# baseline (speedup 1.0000x reference)
"""Multi-head attention (RoPE, causal) on 8 TRN2 NeuronCores.

Sharding: DP2 x TP4. Core c handles batch b = c//4 and heads
H_c = {4*(c%4) .. 4*(c%4)+3}. Attention outputs are exchanged with two
8-rank AllToAlls (bf16, q-sliced), after which every core computes the
final out-projection for a 256-row q-slice of BOTH batches with the full
head dimension locally. No reduction collective; the host-side unshard
is a pure concatenation.

Device pipeline (numpy-validated decomposition; bf16 matmuls with fp32
PSUM accumulation):
  - x^T via X-bar DMA transpose (bf16), issued before all other DMA so
    the first projection chunk starts ASAP.
  - RoPE rotate-every-two as a matmul against a constant +-1
    permutation, then cos/sin multiply-adds on DVE. Rotated K^T is
    written into per-head zero-padded [128, S] tiles so the scores
    matmul runs with K=128 stationary (full-rate PE streaming; K=64
    matmuls stream at half rate).
  - V natural with a ones column folded into augmented Wv/bias (gives
    the softmax denominator for free in the attention matmul); V tiles
    are over-allocated to 324 cols so the AV stationary can be a full
    [128,128] slice (M=65 matmuls also stream at half rate). Garbage
    columns only feed PSUM partitions 65..127, which are never read.
  - softmax without max subtraction (scores ~ N(0,1): exp cannot
    overflow); one fused Exp per k-block covering both heads' scores
    ([128,1024] PSUM tile); division by the denominator after the
    attention matmul via reciprocal_approx_fast (DVE) + GpSimd
    partition_broadcast (no PE broadcast matmul, no PSUM-PSUM copies).
  - causality at block granularity: strictly-above-diagonal blocks
    skipped, diagonal blocks column-sliced, fine triangle masked by a
    [128,128] bf16 multiply on DVE.
  - the k-block loop is software-pipelined (scores for kb+1 issue
    before attnV of kb) to keep the PE stream dense.
"""

import sys

for _p in ("/opt/trn_rl_repo",):
    if _p not in sys.path:
        sys.path.insert(0, _p)

import numpy as np
import ml_dtypes

from concourse import bacc, bass, mybir, tile
from concourse.bass_utils import run_bass_kernel_spmd

F32 = mybir.dt.float32
BF16 = mybir.dt.bfloat16

D, H, HD, S, B = 1024, 16, 64, 2048, 2
HPC = 4          # heads per core
NP = 2           # head pairs per core
QC = 512         # q-chunk size
KB = 128         # k-block size
NQC = S // QC    # 4
NKB = S // KB    # 16
NC = 8           # total cores; the AllToAll spans all 8
SLC = S // NC    # 256 rows of final output per core (for BOTH batches)
VTW = 65 * HPC + 64  # V tile width: 4 slots of 65 + 63 pad + align

Ident = mybir.ActivationFunctionType.Identity
Exp = mybir.ActivationFunctionType.Exp


def _host_constants():
    pos = np.arange(S, dtype=np.float64)
    inv_freq = 1.0 / (10000.0 ** (np.arange(0, HD, 2, dtype=np.float64) / HD))
    freqs = np.outer(pos, inv_freq)
    cosT = np.repeat(np.cos(freqs), 2, axis=1).T.astype(np.float32)  # [64, S]
    sinT = np.repeat(np.sin(freqs), 2, axis=1).T.astype(np.float32)
    # pair-stacked: same table on both 64-partition halves
    cosT = np.concatenate([cosT, cosT], axis=0)  # [128, S]
    sinT = np.concatenate([sinT, sinT], axis=0)
    perm = np.zeros((128, 128), dtype=np.float32)
    for base in (0, 64):
        for i in range(32):
            perm[base + 2 * i + 1, base + 2 * i] = -1.0
            perm[base + 2 * i, base + 2 * i + 1] = 1.0
    # causal fine triangle for a 128-col diagonal slice: keep q >= k
    tri = (np.arange(128)[None, :] >= np.arange(128)[:, None]).astype(np.float32)
    return cosT, sinT, perm, tri


def build_program(debug=False):
    cosT, sinT, perm_np, tri_np = _host_constants()

    nc = bacc.Bacc(None, target_bir_lowering=False)
    dbg = {}
    if debug:
        dbg["ktp"] = nc.declare_dram_parameter("d_ktp", [128, 1024], F32, isOutput=True)
        dbg["qt"] = nc.declare_dram_parameter("d_qt", [128, 512], F32, isOutput=True)
        dbg["pt"] = nc.declare_dram_parameter("d_pt", [128, 1024], F32, isOutput=True)
        dbg["av"] = nc.declare_dram_parameter("d_av", [128, 512], F32, isOutput=True)
        dbg["rc"] = nc.declare_dram_parameter("d_rc", [1, 512], F32, isOutput=True)
        dbg["bc"] = nc.declare_dram_parameter("d_bc", [64, 512], F32, isOutput=True)
        dbg["at"] = nc.declare_dram_parameter("d_at", [64, 512], F32, isOutput=True)
        dbg["vt"] = nc.declare_dram_parameter("d_vt", [128, VTW], F32, isOutput=True)

    # --- I/O ---------------------------------------------------------
    xb = nc.declare_dram_parameter("xb", [S, D], BF16, isOutput=False)
    # weights arrive pre-rearranged from the host (partition-major) so the
    # loads are simple contiguous DMAs — the (c p) n -> p c n rearrange on
    # the DMA path costs ~1000 descriptors each on the Sync sequencer.
    wq = nc.declare_dram_parameter("wq", [128, 8, 256], BF16, isOutput=False)
    wk = nc.declare_dram_parameter("wk", [128, 8, 256], BF16, isOutput=False)
    wv = nc.declare_dram_parameter("wv", [128, 8, 260], BF16, isOutput=False)
    bq = nc.declare_dram_parameter("bq", [128, NP], F32, isOutput=False)
    bk = nc.declare_dram_parameter("bk", [128, NP], F32, isOutput=False)
    bv = nc.declare_dram_parameter("bv", [1, 260], BF16, isOutput=False)
    ones = nc.declare_dram_parameter("ones", [128, 128], BF16, isOutput=False)
    perm = nc.declare_dram_parameter("perm", [128, 128], BF16, isOutput=False)
    wout = nc.declare_dram_parameter("wout", [128, 8, D], BF16, isOutput=False)
    bout = nc.declare_dram_parameter("bout", [1, D], BF16, isOutput=False)
    out = nc.declare_dram_parameter("out_s", [B, SLC, D], BF16, isOutput=True)

    cos_c = nc.inline_tensor(cosT.astype(ml_dtypes.bfloat16), name="cos_c")
    sin_c = nc.inline_tensor(sinT.astype(ml_dtypes.bfloat16), name="sin_c")
    tri_c = nc.inline_tensor(tri_np.astype(ml_dtypes.bfloat16), name="tri_c")

    with tile.TileContext(nc) as tc:
        with (
            tc.tile_pool(name="persist", bufs=1) as pp,
            tc.tile_pool(name="dram", bufs=1, space="DRAM") as dp,
        ):
            # x^T: [128, 8*2048], d-chunk dc at cols [2048*dc, +2048).
            # The X-bar transposes cost ~1.3us each on the Sync sequencer,
            # so interleave them with the weight loads: sc0's transposes
            # first, then the first-needed weights, then the rest.
            xt = pp.tile([128, 8 * S], BF16)

            def issue_transposes(sc, dcs=range(8)):
                for dc in dcs:
                    nc.sync.dma_start(
                        out=xt[:, S * dc + QC * sc : S * dc + QC * sc + QC],
                        in_=xb[QC * sc : QC * sc + QC, 128 * dc : 128 * dc + 128],
                        transpose=True,
                    )

            # The projection's c-accumulation only needs d-chunk c at step c,
            # so the first matmul can start after ONE transpose + wq. Split
            # the transpose issue around the weight loads (everything shares
            # the serial Sync dispatch queue).
            issue_transposes(0, range(0, 2))

            wq_s = pp.tile([128, 8, 256], BF16)
            wk_s = pp.tile([128, 8, 256], BF16)
            nc.sync.dma_start(out=wq_s[:], in_=wq[:])
            nc.sync.dma_start(out=wk_s[:], in_=wk[:])
            bq_s = pp.tile([128, NP], F32)
            bk_s = pp.tile([128, NP], F32)
            nc.sync.dma_start(out=bq_s[:], in_=bq[:])
            nc.sync.dma_start(out=bk_s[:], in_=bk[:])

            issue_transposes(0, range(2, 5))

            perm_s = pp.tile([128, 128], BF16)
            nc.sync.dma_start(out=perm_s[:], in_=perm[:])
            cos_s = pp.tile([128, S], BF16)
            sin_s = pp.tile([128, S], BF16)
            nc.sync.dma_start(out=cos_s[:], in_=cos_c[:])
            nc.sync.dma_start(out=sin_s[:], in_=sin_c[:])

            issue_transposes(0, range(5, 8))
            issue_transposes(1)

            wv_s = pp.tile([128, 8, 260], BF16)
            nc.sync.dma_start(out=wv_s[:], in_=wv[:])
            bv_s = pp.tile([1, 260], BF16)
            nc.sync.dma_start(out=bv_s[:], in_=bv[:])
            ones_f = pp.tile([128, 128], BF16)
            nc.sync.dma_start(out=ones_f[:], in_=ones[:])
            tri_s = pp.tile([128, 128], BF16)
            nc.sync.dma_start(out=tri_s[:], in_=tri_c[:])

            issue_transposes(2)
            issue_transposes(3)

            # phase-C weights: needed only at ~200us but issued now so their
            # descriptor-gen/transfer never gates the out-projection.
            wo_s = pp.tile([128, 8, D], BF16)
            nc.sync.dma_start(out=wo_s[:], in_=wout[:])
            bo_s = pp.tile([1, D], BF16)
            nc.sync.dma_start(out=bo_s[:], in_=bout[:])

            # dummy broadcast: forces the gpsimd 'attn' library load to happen
            # here (idle startup) instead of stalling the first softmax
            # epilogue mid-kernel with a ~7us LIBRARY_RELOAD.
            warm_i = pp.tile([1, 16], F32)
            warm_o = pp.tile([64, 16], F32)
            nc.gpsimd.memset(warm_i[:], 1.0)
            nc.gpsimd.partition_broadcast(warm_o[:], warm_i[:])

            # persistent activations
            qt = pp.tile([128, NP * S], BF16)   # rotated Q^T, pair-major
            # rotated K^T per head, zero-padded to 128 partitions so the
            # scores matmul gets a full-K stationary. Head 2p+h occupies
            # rows [64h, 64h+64) (matching qt's pair-stacked layout);
            # the other 64 rows stay zero.
            ktp = [pp.tile([128, S], BF16, name=f"ktp{h}") for h in range(HPC)]
            for h in range(HPC):
                nc.gpsimd.memset(ktp[h][:], 0.0)
            vt = [pp.tile([128, VTW], BF16, name=f"vt{i}") for i in range(NKB)]
            for i in range(NKB):
                # tail beyond the projected 260 cols must be finite: it is
                # read (x0 weight) by the M-padded AV stationary slices.
                nc.gpsimd.memset(vt[i][:, 260:VTW], 0.0)
            # attnT[p]: [64, 2*S] — within-pair head h at cols [S*h, S*(h+1))
            attnT = [pp.tile([64, NP * S], BF16, name=f"attnT{p}") for p in range(NP)]

            # DRAM bounce buffers for the per-pair 8-rank AllToAll
            cc_in = [
                dp.tile([NC, 128, SLC], BF16, name=f"cc_in{p}") for p in range(NP)
            ]
            cc_out = [
                dp.tile([NC, 128, SLC], BF16, name=f"cc_out{p}") for p in range(NP)
            ]

            # =============================================================
            # Phase A: QKV projection, RoPE
            # =============================================================
            with (
                tc.tile_pool(name="qkraw", bufs=3) as rawp,
                tc.tile_pool(name="pj_psum", bufs=3, space="PSUM") as pjp,
                tc.tile_pool(name="rp_psum", bufs=2, space="PSUM") as rpp,
            ):
                # QKV projection + RoPE, chunk-wise, s-chunk outer so it
                # pipelines behind the transposes.
                for sc in range(NQC):
                    ssl = slice(QC * sc, QC * sc + QC)
                    for p in range(NP):
                        for w_s, b_s, is_k in (
                            (wq_s, bq_s, False),
                            (wk_s, bk_s, True),
                        ):
                            ps = pjp.tile([128, 512], F32, tag="pj")
                            for c in range(8):
                                nc.tensor.matmul(
                                    ps[:],
                                    w_s[:, c, 128 * p : 128 * p + 128],
                                    xt[:, S * c + QC * sc : S * c + QC * sc + QC],
                                    start=(c == 0),
                                    stop=(c == 7),
                                )
                            raw = rawp.tile([128, 512], BF16, tag="raw")
                            nc.scalar.activation(
                                raw[:], ps[:], Ident, bias=b_s[:, p : p + 1]
                            )
                            pr = rpp.tile([128, 512], F32, tag="rp")
                            nc.tensor.matmul(
                                pr[:], perm_s[:], raw[:], start=True, stop=True
                            )
                            if is_k:
                                tcos = rawp.tile([128, 512], BF16, tag="tcos")
                                tsin = rawp.tile([128, 512], BF16, tag="tsin")
                                nc.vector.tensor_mul(tcos[:], raw[:], cos_s[:, ssl])
                                nc.vector.tensor_mul(tsin[:], pr[:], sin_s[:, ssl])
                                nc.vector.tensor_add(
                                    ktp[2 * p][0:64, ssl], tcos[0:64], tsin[0:64]
                                )
                                nc.vector.tensor_add(
                                    ktp[2 * p + 1][64:128, ssl],
                                    tcos[64:128],
                                    tsin[64:128],
                                )
                            else:
                                dst = qt[:, S * p + QC * sc : S * p + QC * sc + QC]
                                rtmp = rawp.tile([128, 512], BF16, tag="rtmp")
                                nc.vector.tensor_mul(dst, raw[:], cos_s[:, ssl])
                                nc.vector.tensor_mul(rtmp[:], pr[:], sin_s[:, ssl])
                                nc.vector.tensor_add(dst, dst, rtmp[:])
                    # V natural [s, 4*65] for the 4 s-blocks of this chunk
                    for sb in range(4 * sc, 4 * sc + 4):
                        ps = pjp.tile([128, 260], F32, tag="pj")
                        for c in range(8):
                            nc.tensor.matmul(
                                ps[:],
                                xt[:, S * c + 128 * sb : S * c + 128 * sb + 128],
                                wv_s[:, c, :],
                                start=(c == 0),
                                stop=False,
                            )
                        nc.tensor.matmul(
                            ps[:],
                            ones_f[0:1, 0:128],
                            bv_s[:],
                            start=False,
                            stop=True,
                        )
                        nc.vector.tensor_copy(vt[sb][:, 0:260], ps[:])

            if debug:
                with tc.tile_pool(name="dbgp", bufs=1) as dbp:
                    dk = dbp.tile([128, 1024], F32, name="dk")
                    nc.vector.tensor_copy(dk[:, 0:512], ktp[0][:, 0:512])
                    nc.vector.tensor_copy(dk[:, 512:1024], ktp[1][:, 0:512])
                    nc.sync.dma_start(out=dbg["ktp"][:], in_=dk[:])
                    dq = dbp.tile([128, 512], F32, name="dq")
                    nc.vector.tensor_copy(dq[:], qt[:, 0:512])
                    nc.sync.dma_start(out=dbg["qt"][:], in_=dq[:])
                    dv = dbp.tile([128, VTW], F32, name="dv")
                    nc.vector.tensor_copy(dv[:], vt[0][:])
                    nc.sync.dma_start(out=dbg["vt"][:], in_=dv[:])

            # =============================================================
            # Phase B: attention per (pair, q-chunk); both heads fused in
            # one [128,1024] scores tile / one Exp; k-block loop software-
            # pipelined (depth 1) to keep the PE stream dense.
            # =============================================================
            with (
                tc.tile_pool(name="p_pool", bufs=4) as ppool,
                tc.tile_pool(name="recip", bufs=4) as rcp,
                tc.tile_pool(name="bcast", bufs=4) as bcp,
                tc.tile_pool(name="sc_psum", bufs=3, space="PSUM") as scp,
                tc.tile_pool(name="av_psum", bufs=2, space="PSUM") as avp,
            ):
                for p in range(NP):
                    for qc in range(NQC):
                        nkb_q = 4 * qc + 4
                        av = [
                            avp.tile([128, 512], F32, tag="av", name=f"av{_h}")
                            for _h in range(2)
                        ]

                        def emit_scores(kb, p=p, qc=qc):
                            mrel = kb - 4 * qc
                            c0 = 128 * max(mrel, 0)  # first valid q-col
                            sc_ps = scp.tile([128, 1024], F32, tag="sc")
                            for h in range(2):
                                nc.tensor.matmul(
                                    sc_ps[:, 512 * h + c0 : 512 * h + 512],
                                    ktp[2 * p + h][:, KB * kb : KB * kb + KB],
                                    qt[
                                        :,
                                        S * p + QC * qc + c0 : S * p + QC * qc + 512,
                                    ],
                                    start=True,
                                    stop=True,
                                )
                            p_t = ppool.tile([128, 1024], BF16, tag="p")
                            if c0 == 0:
                                nc.scalar.activation(
                                    p_t[:], sc_ps[:], Exp, scale=float(HD**-0.5)
                                )
                            else:
                                for h in range(2):
                                    nc.scalar.activation(
                                        p_t[:, 512 * h + c0 : 512 * h + 512],
                                        sc_ps[:, 512 * h + c0 : 512 * h + 512],
                                        Exp,
                                        scale=float(HD**-0.5),
                                    )
                            if mrel >= 0:
                                for h in range(2):
                                    nc.vector.tensor_mul(
                                        p_t[:, 512 * h + c0 : 512 * h + c0 + 128],
                                        p_t[:, 512 * h + c0 : 512 * h + c0 + 128],
                                        tri_s[:],
                                    )
                            if debug and p == 0 and qc == 0 and kb == 0:
                                dpt = rcp.tile([128, 1024], F32, tag="dpt")
                                nc.vector.tensor_copy(dpt[:], p_t[:])
                                nc.sync.dma_start(out=dbg["pt"][:], in_=dpt[:])
                            return p_t, c0

                        def emit_av(kb, pt_c0, p=p, nkb_q=nkb_q):
                            p_t, c0 = pt_c0
                            for h in range(2):
                                nc.tensor.matmul(
                                    av[h][:, c0:512],
                                    vt[kb][
                                        :, 65 * (2 * p + h) : 65 * (2 * p + h) + 128
                                    ],
                                    p_t[:, 512 * h + c0 : 512 * h + 512],
                                    start=(kb == 0),
                                    stop=(kb == nkb_q - 1),
                                )

                        pipe = []
                        for kb in range(nkb_q):
                            pipe.append((kb, emit_scores(kb)))
                            if len(pipe) > 2:
                                emit_av(*pipe.pop(0))
                        for item in pipe:
                            emit_av(*item)

                        for h in range(2):
                            # copy av out of PSUM immediately so the psum
                            # slot recycles fast (next qc's AV accumulation),
                            # then normalize from the SBUF copies.
                            # reciprocal_approx_fast (custom DVE op) misreads
                            # inputs based at partition 64, so the denominator
                            # row is staged to its own partition-0 tile.
                            avs = rcp.tile([64, 512], F32, tag="avs")
                            dn = rcp.tile([1, 512], F32, tag="dn")
                            nc.vector.tensor_copy(avs[:], av[h][0:64, :])
                            nc.vector.tensor_copy(dn[:], av[h][64:65, :])
                            rc = rcp.tile([1, 512], F32, tag="rc")
                            nc.vector.reciprocal_approx_fast(out=rc[:], in_=dn[:])
                            bc = bcp.tile([64, 512], F32, tag="bc")
                            nc.gpsimd.partition_broadcast(bc[:], rc[:])
                            nc.vector.tensor_mul(
                                attnT[p][:, S * h + QC * qc : S * h + QC * qc + QC],
                                avs[:],
                                bc[:],
                            )
                        # ship this q-chunk's slice of attnT right away: the
                        # A2A source groups 2qc, 2qc+1 cover q rows
                        # [512qc, 512qc+512) for both heads.
                        for h in range(2):
                            nc.sync.dma_start(
                                out=cc_in[p].rearrange("g p q -> p g q")[
                                    64 * h : 64 * h + 64, 2 * qc : 2 * qc + 2
                                ],
                                in_=attnT[p][
                                    :, S * h + QC * qc : S * h + QC * qc + QC
                                ].rearrange("p (g q) -> p g q", g=2),
                            )
                    nc.gpsimd.collective_compute(
                        "AllToAll",
                        mybir.AluOpType.bypass,
                        ins=[cc_in[p].opt()],
                        outs=[cc_out[p].opt()],
                        replica_groups=[[0, 1, 2, 3, 4, 5, 6, 7]],
                    )

            # =============================================================
            # Phase C: gathered attn^T -> out projection for my q-slice.
            # Even chunks (pair 0) arrive one AllToAll earlier, so their
            # accumulation overlaps the second AllToAll.
            # =============================================================
            with (
                tc.tile_pool(name="af_pool", bufs=1) as afp,
                tc.tile_pool(name="out_sb", bufs=4) as osp,
                tc.tile_pool(name="op_psum", bufs=8, space="PSUM") as opp,
            ):
                # af[b2][k]: head-dim chunk k (rows [128k, +128) of attn for
                # batch b2) over my SLC q-rows; source core 4*b2 + k//2,
                # pair k%2.
                af = [
                    [afp.tile([128, SLC], BF16, name=f"af{b2}_{k}") for k in range(8)]
                    for b2 in range(B)
                ]
                slots = []  # (psum, b2, sb, nsl)
                for b2 in range(B):
                    for sb in range(SLC // 128):
                        for nc2 in range(2):
                            nsl = slice(512 * nc2, 512 * nc2 + 512)
                            ps = opp.tile(
                                [128, 512], F32, tag="op", name=f"op{b2}{sb}{nc2}"
                            )
                            slots.append((ps, b2, sb, nsl))

                for p in range(NP):
                    for src in range(NC):
                        b2, g = src // 4, src % 4
                        nc.sync.dma_start(out=af[b2][2 * g + p][:], in_=cc_out[p][src])
                    for ps, b2, sb, nsl in slots:
                        if p == 0:
                            nc.tensor.matmul(
                                ps[:],
                                ones_f[0:1, 0:128],
                                bo_s[:, nsl],
                                start=True,
                                stop=False,
                            )
                        for k in range(p, 8, 2):
                            nc.tensor.matmul(
                                ps[:],
                                af[b2][k][:, 128 * sb : 128 * sb + 128],
                                wo_s[:, k, nsl],
                                start=False,
                                stop=(p == 1 and k == 7),
                            )
                        if p == 1:
                            # drain this slot immediately (Act engine is idle
                            # here; overlaps the remaining slots' matmuls)
                            o_t = osp.tile([128, 512], BF16, tag="o")
                            nc.scalar.activation(o_t[:], ps[:], Ident)
                            nc.sync.dma_start(
                                out=out[b2, 128 * sb : 128 * sb + 128, nsl],
                                in_=o_t[:],
                            )
    nc.finalize()
    return nc


_PROGRAM = None


def _get_program():
    global _PROGRAM
    if _PROGRAM is None:
        _PROGRAM = build_program()
    return _PROGRAM


def make_in_maps(x, Wqkv, bqkv, Wout, bout):
    x = np.asarray(x, dtype=np.float32)
    Wqkv = np.asarray(Wqkv, dtype=np.float32)
    bqkv = np.asarray(bqkv, dtype=np.float32)
    Wout = np.asarray(Wout, dtype=np.float32)
    bout = np.asarray(bout, dtype=np.float32)

    def chunk_major(w):  # [D, n] -> [128, 8, n] (d-chunk-major for matmul lhsT)
        n = w.shape[1]
        return np.ascontiguousarray(
            w.reshape(8, 128, n).transpose(1, 0, 2)
        ).astype(ml_dtypes.bfloat16)

    wout_bf = chunk_major(Wout)
    bout_bf = bout.reshape(1, D).astype(ml_dtypes.bfloat16)
    _, _, perm_np, _ = _host_constants()
    ones_np = np.ones((128, 128), dtype=ml_dtypes.bfloat16)
    in_maps = []
    for c in range(8):
        b, g = c // 4, c % 4
        cols = slice(64 * HPC * g, 64 * HPC * (g + 1))  # this core's head dims
        # V weights augmented with a zero column per head slot; the matching
        # bias element is 1.0, so V tiles come out as [v(64) | 1] per head.
        wv_aug = np.zeros((D, 65 * HPC), dtype=np.float32)
        bv_aug = np.zeros((1, 65 * HPC), dtype=np.float32)
        wv_c = Wqkv[:, 2 * D :][:, cols]
        bv_c = bqkv[2 * D :][cols]
        for h in range(HPC):
            wv_aug[:, 65 * h : 65 * h + 64] = wv_c[:, 64 * h : 64 * h + 64]
            bv_aug[0, 65 * h : 65 * h + 64] = bv_c[64 * h : 64 * h + 64]
            bv_aug[0, 65 * h + 64] = 1.0
        in_maps.append(
            {
                "xb": np.ascontiguousarray(x[:, b, :]).astype(ml_dtypes.bfloat16),
                "wq": chunk_major(np.ascontiguousarray(Wqkv[:, 0 * D :][:, cols])),
                "wk": chunk_major(np.ascontiguousarray(Wqkv[:, 1 * D :][:, cols])),
                "wv": chunk_major(wv_aug),
                "bq": np.ascontiguousarray(bqkv[0 * D :][cols].reshape(NP, 128).T),
                "bk": np.ascontiguousarray(bqkv[1 * D :][cols].reshape(NP, 128).T),
                "bv": bv_aug.astype(ml_dtypes.bfloat16),
                "ones": ones_np,
                "perm": perm_np.astype(ml_dtypes.bfloat16),
                "wout": wout_bf,
                "bout": bout_bf,
            }
        )
    return in_maps


def unshard(results):
    out = np.empty((S, B, D), dtype=np.float32)
    for r in range(8):
        for b2 in range(B):
            out[SLC * r : SLC * (r + 1), b2, :] = np.asarray(
                results[r]["out_s"][b2]
            ).astype(np.float32)
    return out


def kernel(x, Wqkv, bqkv, Wout, bout, **_kw):
    nc = _get_program()
    in_maps = make_in_maps(x, Wqkv, bqkv, Wout, bout)
    res = run_bass_kernel_spmd(nc, in_maps, list(range(8)))
    return unshard(res.results)


# revision 24
# speedup vs baseline: 1.0183x; 1.0183x over previous
"""Multi-head attention (RoPE, causal) on 8 TRN2 NeuronCores.

Sharding: DP2 x TP4. Core c handles batch b = c//4 and heads
H_c = {4*(c%4) .. 4*(c%4)+3}. Attention outputs are exchanged with two
8-rank AllToAlls (bf16, q-sliced), after which every core computes the
final out-projection for a 256-row q-slice of BOTH batches with the full
head dimension locally. No reduction collective; the host-side unshard
is a pure concatenation.

Device pipeline (numpy-validated decomposition; bf16 matmuls with fp32
PSUM accumulation):
  - x^T via X-bar DMA transpose (bf16), issued before all other DMA so
    the first projection chunk starts ASAP.
  - RoPE rotate-every-two as a matmul against a constant +-1
    permutation, then cos/sin multiply-adds on DVE. Rotated K^T is
    written into per-head zero-padded [128, S] tiles so the scores
    matmul runs with K=128 stationary (full-rate PE streaming; K=64
    matmuls stream at half rate).
  - V natural with a ones column folded into augmented Wv/bias (gives
    the softmax denominator for free in the attention matmul); V tiles
    are over-allocated to 324 cols so the AV stationary can be a full
    [128,128] slice (M=65 matmuls also stream at half rate). Garbage
    columns only feed PSUM partitions 65..127, which are never read.
  - softmax without max subtraction (scores ~ N(0,1): exp cannot
    overflow); one fused Exp per k-block covering both heads' scores
    ([128,1024] PSUM tile); division by the denominator after the
    attention matmul via reciprocal_approx_fast (DVE) + GpSimd
    partition_broadcast (no PE broadcast matmul, no PSUM-PSUM copies).
  - causality at block granularity: strictly-above-diagonal blocks
    skipped, diagonal blocks column-sliced, fine triangle masked by a
    [128,128] bf16 multiply on DVE.
  - the k-block loop is software-pipelined (scores for kb+1 issue
    before attnV of kb) to keep the PE stream dense.
"""

import sys

for _p in ("/opt/trn_rl_repo",):
    if _p not in sys.path:
        sys.path.insert(0, _p)

import numpy as np
import ml_dtypes

from concourse import bacc, bass, mybir, tile
from concourse.bass_utils import run_bass_kernel_spmd

F32 = mybir.dt.float32
BF16 = mybir.dt.bfloat16

D, H, HD, S, B = 1024, 16, 64, 2048, 2
HPC = 4          # heads per core
NP = 2           # head pairs per core
QC = 512         # q-chunk size
KB = 128         # k-block size
NQC = S // QC    # 4
NKB = S // KB    # 16
NC = 8           # total cores; the AllToAll spans all 8
SLC = S // NC    # 256 rows of final output per core (for BOTH batches)
VTW = 65 * HPC + 64  # V tile width: 4 slots of 65 + 63 pad + align

Ident = mybir.ActivationFunctionType.Identity
Exp = mybir.ActivationFunctionType.Exp


def _host_constants():
    pos = np.arange(S, dtype=np.float64)
    inv_freq = 1.0 / (10000.0 ** (np.arange(0, HD, 2, dtype=np.float64) / HD))
    freqs = np.outer(pos, inv_freq)
    cosT = np.repeat(np.cos(freqs), 2, axis=1).T.astype(np.float32)  # [64, S]
    sinT = np.repeat(np.sin(freqs), 2, axis=1).T.astype(np.float32)
    # pair-stacked: same table on both 64-partition halves
    cosT = np.concatenate([cosT, cosT], axis=0)  # [128, S]
    sinT = np.concatenate([sinT, sinT], axis=0)
    perm = np.zeros((128, 128), dtype=np.float32)
    for base in (0, 64):
        for i in range(32):
            perm[base + 2 * i + 1, base + 2 * i] = -1.0
            perm[base + 2 * i, base + 2 * i + 1] = 1.0
    # causal fine triangle for a 128-col diagonal slice: keep q >= k
    tri = (np.arange(128)[None, :] >= np.arange(128)[:, None]).astype(np.float32)
    return cosT, sinT, perm, tri


def build_program(debug=False):
    cosT, sinT, perm_np, tri_np = _host_constants()

    nc = bacc.Bacc(None, target_bir_lowering=False)
    dbg = {}
    if debug:
        dbg["ktp"] = nc.declare_dram_parameter("d_ktp", [128, 1024], F32, isOutput=True)
        dbg["qt"] = nc.declare_dram_parameter("d_qt", [128, 512], F32, isOutput=True)
        dbg["pt"] = nc.declare_dram_parameter("d_pt", [128, 1024], F32, isOutput=True)
        dbg["av"] = nc.declare_dram_parameter("d_av", [128, 512], F32, isOutput=True)
        dbg["rc"] = nc.declare_dram_parameter("d_rc", [1, 512], F32, isOutput=True)
        dbg["bc"] = nc.declare_dram_parameter("d_bc", [64, 512], F32, isOutput=True)
        dbg["at"] = nc.declare_dram_parameter("d_at", [64, 512], F32, isOutput=True)
        dbg["vt"] = nc.declare_dram_parameter("d_vt", [128, VTW], F32, isOutput=True)

    # --- I/O ---------------------------------------------------------
    xb = nc.declare_dram_parameter("xb", [S, D], BF16, isOutput=False)
    # weights arrive pre-rearranged from the host (partition-major) so the
    # loads are simple contiguous DMAs — the (c p) n -> p c n rearrange on
    # the DMA path costs ~1000 descriptors each on the Sync sequencer.
    wq = nc.declare_dram_parameter("wq", [128, 8, 256], BF16, isOutput=False)
    wk = nc.declare_dram_parameter("wk", [128, 8, 256], BF16, isOutput=False)
    wv = nc.declare_dram_parameter("wv", [128, 8, 260], BF16, isOutput=False)
    bq = nc.declare_dram_parameter("bq", [128, NP], F32, isOutput=False)
    bk = nc.declare_dram_parameter("bk", [128, NP], F32, isOutput=False)
    bv = nc.declare_dram_parameter("bv", [1, 260], BF16, isOutput=False)
    ones = nc.declare_dram_parameter("ones", [128, 128], BF16, isOutput=False)
    perm = nc.declare_dram_parameter("perm", [128, 128], BF16, isOutput=False)
    wout = nc.declare_dram_parameter("wout", [128, 8, D], BF16, isOutput=False)
    bout = nc.declare_dram_parameter("bout", [1, D], BF16, isOutput=False)
    out = nc.declare_dram_parameter("out_s", [B, SLC, D], BF16, isOutput=True)

    cos_c = nc.inline_tensor(cosT.astype(ml_dtypes.bfloat16), name="cos_c")
    sin_c = nc.inline_tensor(sinT.astype(ml_dtypes.bfloat16), name="sin_c")
    tri_c = nc.inline_tensor(tri_np.astype(ml_dtypes.bfloat16), name="tri_c")

    with tile.TileContext(nc) as tc:
        with (
            tc.tile_pool(name="persist", bufs=1) as pp,
            tc.tile_pool(name="dram", bufs=1, space="DRAM") as dp,
        ):
            # x^T: [128, 8*2048], d-chunk dc at cols [2048*dc, +2048).
            # The X-bar transposes cost ~1.3us each on the Sync sequencer,
            # so interleave them with the weight loads: sc0's transposes
            # first, then the first-needed weights, then the rest.
            xt = pp.tile([128, 8 * S], BF16)

            def issue_transposes(sc, dcs=range(8)):
                for dc in dcs:
                    nc.sync.dma_start(
                        out=xt[:, S * dc + QC * sc : S * dc + QC * sc + QC],
                        in_=xb[QC * sc : QC * sc + QC, 128 * dc : 128 * dc + 128],
                        transpose=True,
                    )

            issue_transposes(0)

            wq_s = pp.tile([128, 8, 256], BF16)
            wk_s = pp.tile([128, 8, 256], BF16)
            nc.sync.dma_start(out=wq_s[:], in_=wq[:])
            nc.sync.dma_start(out=wk_s[:], in_=wk[:])
            bq_s = pp.tile([128, NP], F32)
            bk_s = pp.tile([128, NP], F32)
            nc.sync.dma_start(out=bq_s[:], in_=bq[:])
            nc.sync.dma_start(out=bk_s[:], in_=bk[:])
            perm_s = pp.tile([128, 128], BF16)
            nc.sync.dma_start(out=perm_s[:], in_=perm[:])
            cos_s = pp.tile([128, S], BF16)
            sin_s = pp.tile([128, S], BF16)
            nc.sync.dma_start(out=cos_s[:], in_=cos_c[:])
            nc.sync.dma_start(out=sin_s[:], in_=sin_c[:])

            issue_transposes(1)

            wv_s = pp.tile([128, 8, 260], BF16)
            nc.sync.dma_start(out=wv_s[:], in_=wv[:])
            bv_s = pp.tile([1, 260], BF16)
            nc.sync.dma_start(out=bv_s[:], in_=bv[:])
            ones_f = pp.tile([128, 128], BF16)
            nc.sync.dma_start(out=ones_f[:], in_=ones[:])
            tri_s = pp.tile([128, 128], BF16)
            nc.sync.dma_start(out=tri_s[:], in_=tri_c[:])

            issue_transposes(2)
            issue_transposes(3)

            # phase-C weights: needed only at ~200us but issued now so their
            # descriptor-gen/transfer never gates the out-projection.
            wo_s = pp.tile([128, 8, D], BF16)
            nc.sync.dma_start(out=wo_s[:], in_=wout[:])
            bo_s = pp.tile([1, D], BF16)
            nc.sync.dma_start(out=bo_s[:], in_=bout[:])

            # dummy broadcast: forces the gpsimd 'attn' library load to happen
            # here (idle startup) instead of stalling the first softmax
            # epilogue mid-kernel with a ~7us LIBRARY_RELOAD.
            warm_i = pp.tile([1, 16], F32)
            warm_o = pp.tile([64, 16], F32)
            nc.gpsimd.memset(warm_i[:], 1.0)
            nc.gpsimd.partition_broadcast(warm_o[:], warm_i[:])

            # persistent activations
            qt = pp.tile([128, NP * S], BF16)   # rotated Q^T, pair-major
            # rotated K^T per head, zero-padded to 128 partitions so the
            # scores matmul gets a full-K stationary. Head 2p+h occupies
            # rows [64h, 64h+64) (matching qt's pair-stacked layout);
            # the other 64 rows stay zero.
            ktp = [pp.tile([128, S], BF16, name=f"ktp{h}") for h in range(HPC)]
            for h in range(HPC):
                nc.gpsimd.memset(ktp[h][:], 0.0)
            vt = [pp.tile([128, VTW], BF16, name=f"vt{i}") for i in range(NKB)]
            for i in range(NKB):
                # tail beyond the projected 260 cols must be finite: it is
                # read (x0 weight) by the M-padded AV stationary slices.
                nc.gpsimd.memset(vt[i][:, 260:VTW], 0.0)
            # attnT[p]: [64, 2*S] — within-pair head h at cols [S*h, S*(h+1))
            attnT = [pp.tile([64, NP * S], BF16, name=f"attnT{p}") for p in range(NP)]

            # DRAM bounce buffers for the per-pair 8-rank AllToAll
            cc_in = [
                dp.tile([NC, 128, SLC], BF16, name=f"cc_in{p}") for p in range(NP)
            ]
            cc_out = [
                dp.tile([NC, 128, SLC], BF16, name=f"cc_out{p}") for p in range(NP)
            ]

            # =============================================================
            # Phase A: QKV projection, RoPE
            # =============================================================
            with (
                tc.tile_pool(name="qkraw", bufs=3) as rawp,
                tc.tile_pool(name="pj_psum", bufs=3, space="PSUM") as pjp,
                tc.tile_pool(name="rp_psum", bufs=2, space="PSUM") as rpp,
            ):
                # QKV projection + RoPE, chunk-wise, s-chunk outer so it
                # pipelines behind the transposes.
                for sc in range(NQC):
                    ssl = slice(QC * sc, QC * sc + QC)
                    for p in range(NP):
                        for w_s, b_s, is_k in (
                            (wq_s, bq_s, False),
                            (wk_s, bk_s, True),
                        ):
                            ps = pjp.tile([128, 512], F32, tag="pj")
                            for c in range(8):
                                nc.tensor.matmul(
                                    ps[:],
                                    w_s[:, c, 128 * p : 128 * p + 128],
                                    xt[:, S * c + QC * sc : S * c + QC * sc + QC],
                                    start=(c == 0),
                                    stop=(c == 7),
                                )
                            raw = rawp.tile([128, 512], BF16, tag="raw")
                            nc.scalar.activation(
                                raw[:], ps[:], Ident, bias=b_s[:, p : p + 1]
                            )
                            pr = rpp.tile([128, 512], F32, tag="rp")
                            nc.tensor.matmul(
                                pr[:], perm_s[:], raw[:], start=True, stop=True
                            )
                            if is_k:
                                tcos = rawp.tile([128, 512], BF16, tag="tcos")
                                tsin = rawp.tile([128, 512], BF16, tag="tsin")
                                nc.vector.tensor_mul(tcos[:], raw[:], cos_s[:, ssl])
                                nc.vector.tensor_mul(tsin[:], pr[:], sin_s[:, ssl])
                                nc.vector.tensor_add(
                                    ktp[2 * p][0:64, ssl], tcos[0:64], tsin[0:64]
                                )
                                nc.vector.tensor_add(
                                    ktp[2 * p + 1][64:128, ssl],
                                    tcos[64:128],
                                    tsin[64:128],
                                )
                            else:
                                dst = qt[:, S * p + QC * sc : S * p + QC * sc + QC]
                                rtmp = rawp.tile([128, 512], BF16, tag="rtmp")
                                nc.vector.tensor_mul(dst, raw[:], cos_s[:, ssl])
                                nc.vector.tensor_mul(rtmp[:], pr[:], sin_s[:, ssl])
                                nc.vector.tensor_add(dst, dst, rtmp[:])
                    # V natural [s, 4*65] for the 4 s-blocks of this chunk
                    for sb in range(4 * sc, 4 * sc + 4):
                        ps = pjp.tile([128, 260], F32, tag="pj")
                        for c in range(8):
                            nc.tensor.matmul(
                                ps[:],
                                xt[:, S * c + 128 * sb : S * c + 128 * sb + 128],
                                wv_s[:, c, :],
                                start=(c == 0),
                                stop=False,
                            )
                        nc.tensor.matmul(
                            ps[:],
                            ones_f[0:1, 0:128],
                            bv_s[:],
                            start=False,
                            stop=True,
                        )
                        nc.vector.tensor_copy(vt[sb][:, 0:260], ps[:])

            if debug:
                with tc.tile_pool(name="dbgp", bufs=1) as dbp:
                    dk = dbp.tile([128, 1024], F32, name="dk")
                    nc.vector.tensor_copy(dk[:, 0:512], ktp[0][:, 0:512])
                    nc.vector.tensor_copy(dk[:, 512:1024], ktp[1][:, 0:512])
                    nc.sync.dma_start(out=dbg["ktp"][:], in_=dk[:])
                    dq = dbp.tile([128, 512], F32, name="dq")
                    nc.vector.tensor_copy(dq[:], qt[:, 0:512])
                    nc.sync.dma_start(out=dbg["qt"][:], in_=dq[:])
                    dv = dbp.tile([128, VTW], F32, name="dv")
                    nc.vector.tensor_copy(dv[:], vt[0][:])
                    nc.sync.dma_start(out=dbg["vt"][:], in_=dv[:])

            # =============================================================
            # Phase B: attention per (pair, q-chunk); both heads fused in
            # one [128,1024] scores tile / one Exp; k-block loop software-
            # pipelined (depth 1) to keep the PE stream dense.
            # =============================================================
            with (
                tc.tile_pool(name="p_pool", bufs=4) as ppool,
                tc.tile_pool(name="recip", bufs=4) as rcp,
                tc.tile_pool(name="bcast", bufs=4) as bcp,
                tc.tile_pool(name="sc_psum", bufs=3, space="PSUM") as scp,
                tc.tile_pool(name="av_psum", bufs=2, space="PSUM") as avp,
            ):
                for p in range(NP):
                    for qc in range(NQC):
                        nkb_q = 4 * qc + 4
                        av = [
                            avp.tile([128, 512], F32, tag="av", name=f"av{_h}")
                            for _h in range(2)
                        ]

                        def emit_scores(kb, p=p, qc=qc):
                            mrel = kb - 4 * qc
                            c0 = 128 * max(mrel, 0)  # first valid q-col
                            sc_ps = scp.tile([128, 1024], F32, tag="sc")
                            for h in range(2):
                                nc.tensor.matmul(
                                    sc_ps[:, 512 * h + c0 : 512 * h + 512],
                                    ktp[2 * p + h][:, KB * kb : KB * kb + KB],
                                    qt[
                                        :,
                                        S * p + QC * qc + c0 : S * p + QC * qc + 512,
                                    ],
                                    start=True,
                                    stop=True,
                                )
                            p_t = ppool.tile([128, 1024], BF16, tag="p")
                            if c0 == 0:
                                nc.scalar.activation(
                                    p_t[:], sc_ps[:], Exp, scale=float(HD**-0.5)
                                )
                            else:
                                for h in range(2):
                                    nc.scalar.activation(
                                        p_t[:, 512 * h + c0 : 512 * h + 512],
                                        sc_ps[:, 512 * h + c0 : 512 * h + 512],
                                        Exp,
                                        scale=float(HD**-0.5),
                                    )
                            if mrel >= 0:
                                for h in range(2):
                                    nc.vector.tensor_mul(
                                        p_t[:, 512 * h + c0 : 512 * h + c0 + 128],
                                        p_t[:, 512 * h + c0 : 512 * h + c0 + 128],
                                        tri_s[:],
                                    )
                            if debug and p == 0 and qc == 0 and kb == 0:
                                dpt = rcp.tile([128, 1024], F32, tag="dpt")
                                nc.vector.tensor_copy(dpt[:], p_t[:])
                                nc.sync.dma_start(out=dbg["pt"][:], in_=dpt[:])
                            return p_t, c0

                        def emit_av(kb, pt_c0, p=p, nkb_q=nkb_q):
                            p_t, c0 = pt_c0
                            for h in range(2):
                                nc.tensor.matmul(
                                    av[h][:, c0:512],
                                    vt[kb][
                                        :, 65 * (2 * p + h) : 65 * (2 * p + h) + 128
                                    ],
                                    p_t[:, 512 * h + c0 : 512 * h + 512],
                                    start=(kb == 0),
                                    stop=(kb == nkb_q - 1),
                                )

                        pipe = []
                        for kb in range(nkb_q):
                            pipe.append((kb, emit_scores(kb)))
                            if len(pipe) > 2:
                                emit_av(*pipe.pop(0))
                        for item in pipe:
                            emit_av(*item)

                        for h in range(2):
                            # copy av out of PSUM immediately so the psum
                            # slot recycles fast (next qc's AV accumulation),
                            # then normalize from the SBUF copies.
                            # reciprocal_approx_fast (custom DVE op) misreads
                            # inputs based at partition 64, so the denominator
                            # row is staged to its own partition-0 tile.
                            avs = rcp.tile([64, 512], F32, tag="avs")
                            dn = rcp.tile([1, 512], F32, tag="dn")
                            nc.vector.tensor_copy(avs[:], av[h][0:64, :])
                            nc.vector.tensor_copy(dn[:], av[h][64:65, :])
                            rc = rcp.tile([1, 512], F32, tag="rc")
                            nc.vector.reciprocal_approx_fast(out=rc[:], in_=dn[:])
                            bc = bcp.tile([64, 512], F32, tag="bc")
                            nc.gpsimd.partition_broadcast(bc[:], rc[:])
                            nc.vector.tensor_mul(
                                attnT[p][:, S * h + QC * qc : S * h + QC * qc + QC],
                                avs[:],
                                bc[:],
                            )
                        # ship this q-chunk's slice of attnT right away: the
                        # A2A source groups 2qc, 2qc+1 cover q rows
                        # [512qc, 512qc+512) for both heads.
                        for h in range(2):
                            nc.sync.dma_start(
                                out=cc_in[p].rearrange("g p q -> p g q")[
                                    64 * h : 64 * h + 64, 2 * qc : 2 * qc + 2
                                ],
                                in_=attnT[p][
                                    :, S * h + QC * qc : S * h + QC * qc + QC
                                ].rearrange("p (g q) -> p g q", g=2),
                            )
                    nc.gpsimd.collective_compute(
                        "AllToAll",
                        mybir.AluOpType.bypass,
                        ins=[cc_in[p].opt()],
                        outs=[cc_out[p].opt()],
                        replica_groups=[[0, 1, 2, 3, 4, 5, 6, 7]],
                    )

            # =============================================================
            # Phase C: gathered attn^T -> out projection for my q-slice.
            # Even chunks (pair 0) arrive one AllToAll earlier, so their
            # accumulation overlaps the second AllToAll.
            # =============================================================
            with (
                tc.tile_pool(name="af_pool", bufs=1) as afp,
                tc.tile_pool(name="out_sb", bufs=4) as osp,
                tc.tile_pool(name="op_psum", bufs=8, space="PSUM") as opp,
            ):
                # af[b2][k]: head-dim chunk k (rows [128k, +128) of attn for
                # batch b2) over my SLC q-rows; source core 4*b2 + k//2,
                # pair k%2.
                af = [
                    [afp.tile([128, SLC], BF16, name=f"af{b2}_{k}") for k in range(8)]
                    for b2 in range(B)
                ]
                slots = []  # (psum, b2, sb, nsl)
                for b2 in range(B):
                    for sb in range(SLC // 128):
                        for nc2 in range(2):
                            nsl = slice(512 * nc2, 512 * nc2 + 512)
                            ps = opp.tile(
                                [128, 512], F32, tag="op", name=f"op{b2}{sb}{nc2}"
                            )
                            slots.append((ps, b2, sb, nsl))

                for p in range(NP):
                    for src in range(NC):
                        b2, g = src // 4, src % 4
                        nc.sync.dma_start(out=af[b2][2 * g + p][:], in_=cc_out[p][src])
                    for ps, b2, sb, nsl in slots:
                        if p == 0:
                            nc.tensor.matmul(
                                ps[:],
                                ones_f[0:1, 0:128],
                                bo_s[:, nsl],
                                start=True,
                                stop=False,
                            )
                        for k in range(p, 8, 2):
                            nc.tensor.matmul(
                                ps[:],
                                af[b2][k][:, 128 * sb : 128 * sb + 128],
                                wo_s[:, k, nsl],
                                start=False,
                                stop=(p == 1 and k == 7),
                            )
                        if p == 1:
                            # drain this slot immediately (Act engine is idle
                            # here; overlaps the remaining slots' matmuls)
                            o_t = osp.tile([128, 512], BF16, tag="o")
                            nc.scalar.activation(o_t[:], ps[:], Ident)
                            nc.sync.dma_start(
                                out=out[b2, 128 * sb : 128 * sb + 128, nsl],
                                in_=o_t[:],
                            )
    nc.finalize()
    return nc


_PROGRAM = None


def _get_program():
    global _PROGRAM
    if _PROGRAM is None:
        _PROGRAM = build_program()
    return _PROGRAM


def make_in_maps(x, Wqkv, bqkv, Wout, bout):
    x = np.asarray(x, dtype=np.float32)
    Wqkv = np.asarray(Wqkv, dtype=np.float32)
    bqkv = np.asarray(bqkv, dtype=np.float32)
    Wout = np.asarray(Wout, dtype=np.float32)
    bout = np.asarray(bout, dtype=np.float32)

    def chunk_major(w):  # [D, n] -> [128, 8, n] (d-chunk-major for matmul lhsT)
        n = w.shape[1]
        return np.ascontiguousarray(
            w.reshape(8, 128, n).transpose(1, 0, 2)
        ).astype(ml_dtypes.bfloat16)

    wout_bf = chunk_major(Wout)
    bout_bf = bout.reshape(1, D).astype(ml_dtypes.bfloat16)
    _, _, perm_np, _ = _host_constants()
    ones_np = np.ones((128, 128), dtype=ml_dtypes.bfloat16)
    in_maps = []
    for c in range(8):
        b, g = c // 4, c % 4
        cols = slice(64 * HPC * g, 64 * HPC * (g + 1))  # this core's head dims
        # V weights augmented with a zero column per head slot; the matching
        # bias element is 1.0, so V tiles come out as [v(64) | 1] per head.
        wv_aug = np.zeros((D, 65 * HPC), dtype=np.float32)
        bv_aug = np.zeros((1, 65 * HPC), dtype=np.float32)
        wv_c = Wqkv[:, 2 * D :][:, cols]
        bv_c = bqkv[2 * D :][cols]
        for h in range(HPC):
            wv_aug[:, 65 * h : 65 * h + 64] = wv_c[:, 64 * h : 64 * h + 64]
            bv_aug[0, 65 * h : 65 * h + 64] = bv_c[64 * h : 64 * h + 64]
            bv_aug[0, 65 * h + 64] = 1.0
        in_maps.append(
            {
                "xb": np.ascontiguousarray(x[:, b, :]).astype(ml_dtypes.bfloat16),
                "wq": chunk_major(np.ascontiguousarray(Wqkv[:, 0 * D :][:, cols])),
                "wk": chunk_major(np.ascontiguousarray(Wqkv[:, 1 * D :][:, cols])),
                "wv": chunk_major(wv_aug),
                "bq": np.ascontiguousarray(bqkv[0 * D :][cols].reshape(NP, 128).T),
                "bk": np.ascontiguousarray(bqkv[1 * D :][cols].reshape(NP, 128).T),
                "bv": bv_aug.astype(ml_dtypes.bfloat16),
                "ones": ones_np,
                "perm": perm_np.astype(ml_dtypes.bfloat16),
                "wout": wout_bf,
                "bout": bout_bf,
            }
        )
    return in_maps


def unshard(results):
    out = np.empty((S, B, D), dtype=np.float32)
    for r in range(8):
        for b2 in range(B):
            out[SLC * r : SLC * (r + 1), b2, :] = np.asarray(
                results[r]["out_s"][b2]
            ).astype(np.float32)
    return out


def kernel(x, Wqkv, bqkv, Wout, bout, **_kw):
    nc = _get_program()
    in_maps = make_in_maps(x, Wqkv, bqkv, Wout, bout)
    res = run_bass_kernel_spmd(nc, in_maps, list(range(8)))
    return unshard(res.results)
